# revision 10
# baseline (speedup 1.0000x reference)
"""Bahdanau attention TRN2 kernel — data-parallel over batch on 8 NeuronCores.

Shapes (hardcoded from the problem spec):
  dec_hidden [1, 64, 1024] f32, encoder_outputs [64, 2048, 1024] f32,
  mask [64, 1, 2048] i32, W_w [1024, 2048] f32, W_b [1024] f32, v_w [1, 1024] f32.
Returns (context [64, 1, 1024] f32, atten_weights [64, 2048] f32).

Per-core plan (8 batches/core):
  host prep:  bias = dec_hidden @ Wd.T + W_b  (tiny), WeT = We.T in bf16,
              v in bf16, additive mask (-1e10 where masked).
  device:     stream enc [128s x 1024e] tiles; PE-transpose to encT [e,s] (f32r);
              energy^T[d,s] = WeT.T @ encT in bf16 (PSUM f32);
              ACT tanh with per-partition bias (bias[d] per batch);
              score = v.T @ tanh via M=1 PE matmuls; row softmax (DVE+ACT);
              context = atten.T @ enc from the resident natural tiles (f32r).
"""

import os
from contextlib import ExitStack

import ml_dtypes
import numpy as np

import concourse.bass as bass
import concourse.bacc as bacc
import concourse.tile as tile
from concourse import mybir
from concourse.bass_utils import run_bass_kernel_spmd

B, S, D, E = 64, 2048, 1024, 1024
NCORES = 8
BPC = B // NCORES          # batches per core
P = 128                    # partitions
NT = S // P                # 16 s-tiles per batch
SCH = 1024                 # s-chunk (PSUM free dim per energy tile)
NCH = S // SCH             # 2 chunks per batch
TPH = SCH // 2 // P        # 4 transposes per half-chunk stage
KD = D // P                # 8 d-tiles
KE = E // P                # 8 e-tiles

F32 = mybir.dt.float32
F32R = mybir.dt.float32r
BF16 = mybir.dt.bfloat16
AF = mybir.ActivationFunctionType
AX = mybir.AxisListType

# Fallback switch: if float32r misbehaves on HW, set False (transposes and the
# context matmul then run in plain float32 at 2x/4x cycles per row).
USE_F32R = os.environ.get("KERNEL_NO_F32R", "") == ""

TRACE = False
LAST_RESULTS = None  # BassKernelResults of the last run (for test.py)


def _r(ap):
    """View a float32 AP as float32r for fast PE streaming."""
    return ap.bitcast(F32R) if USE_F32R else ap


def build_nc():
    nc = bacc.Bacc("TRN2", target_bir_lowering=False, debug=False)

    enc_in = nc.dram_tensor("enc_in", [BPC, S, E], F32R if USE_F32R else F32, kind="ExternalInput").ap()
    wet_in = nc.dram_tensor("wet_in", [E, D], BF16, kind="ExternalInput").ap()
    bias_in = nc.dram_tensor("bias_in", [BPC, D], F32, kind="ExternalInput").ap()
    vb_in = nc.dram_tensor("vb_in", [1, D], BF16, kind="ExternalInput").ap()
    mneg_in = nc.dram_tensor("mneg_in", [BPC, S], F32, kind="ExternalInput").ap()
    ident_in = nc.dram_tensor("ident_in", [P, P], F32R if USE_F32R else F32, kind="ExternalInput").ap()
    ctx_out = nc.dram_tensor("ctx_out", [BPC, E], F32, kind="ExternalOutput").ap()
    att_out = nc.dram_tensor("att_out", [BPC, S], F32, kind="ExternalOutput").ap()

    with tile.TileContext(nc) as tc, ExitStack() as ctx:
        const = ctx.enter_context(tc.tile_pool(name="const", bufs=1))
        nat_pool = ctx.enter_context(tc.tile_pool(name="nat", bufs=18))
        enct_pool = ctx.enter_context(tc.tile_pool(name="enct", bufs=10))
        tanh_pool = ctx.enter_context(tc.tile_pool(name="tanh", bufs=10))
        row_pool = ctx.enter_context(tc.tile_pool(name="rows", bufs=2))
        mrow_pool = ctx.enter_context(tc.tile_pool(name="mrows", bufs=2))
        wcol_pool = ctx.enter_context(tc.tile_pool(name="wcol", bufs=2))
        tiny_pool = ctx.enter_context(tc.tile_pool(name="tiny", bufs=8))
        dram_pool = ctx.enter_context(tc.tile_pool(name="dscratch", bufs=2, space="DRAM"))
        pe_psum = ctx.enter_context(tc.tile_pool(name="pe_psum", bufs=2, space="PSUM"))
        tr_psum = ctx.enter_context(tc.tile_pool(name="tr_psum", bufs=2, space="PSUM"))
        v_psum = ctx.enter_context(tc.tile_pool(name="v_psum", bufs=2, space="PSUM"))

        # ---- constants ----
        ident = const.tile([P, P], F32R if USE_F32R else F32)
        nc.sync.dma_start(ident[:], ident_in)
        identr = ident[:]

        weT = const.tile([P, KE * D], BF16)       # block k: WeT[128k:128k+128, :]
        for k in range(KE):
            nc.sync.dma_start(weT[:, k * D:(k + 1) * D], wet_in[k * P:(k + 1) * P, :])

        vT = const.tile([P, KD], BF16)            # col m = v[128m : 128m+128]
        nc.sync.dma_start(vT[:], vb_in.rearrange("a (m p) -> p (a m)", p=P))

        biasT = const.tile([P, BPC * KD], F32)    # col b*KD+m = bias[b, 128m:128m+128]
        nc.sync.dma_start(biasT[:], bias_in.rearrange("b (m p) -> p (b m)", p=P))

        # ---- main loop over this core's batches ----
        for b in range(BPC):
            nat_tiles = []
            score_row = row_pool.tile([1, S], F32)

            for c in range(NCH):
                # stage A: load naturals for this chunk
                for j in range(SCH // P):
                    t = c * (SCH // P) + j
                    nat = nat_pool.tile([P, E], F32R if USE_F32R else F32)
                    nc.sync.dma_start(nat[:], enc_in[b, t * P:(t + 1) * P, :])
                    nat_tiles.append(nat)

                # stage B: PE-transpose this chunk into encT (bf16 on copy-out)
                enct_tiles = []
                for k in range(KE):
                    enct = enct_pool.tile([P, SCH], BF16)
                    for h in range(2):
                        stage = tr_psum.tile([P, 512], F32R if USE_F32R else F32)
                        for j in range(TPH):
                            t = c * (SCH // P) + h * TPH + j
                            nc.tensor.matmul(
                                stage[:, j * P:(j + 1) * P],
                                nat_tiles[t][:, k * P:(k + 1) * P],
                                identr,
                                is_transpose=True,
                                start=(j == 0),
                                stop=(j == TPH - 1),
                            )
                        if (k + h) % 2 == 0:
                            nc.vector.tensor_copy(enct[:, h * 512:(h + 1) * 512], stage[:])
                        else:
                            nc.scalar.copy(enct[:, h * 512:(h + 1) * 512], stage[:])
                    enct_tiles.append(enct)

                # stage C: energy matmul + fused bias+tanh per d-tile
                tanh_tiles = []
                for m in range(KD):
                    pe = pe_psum.tile([P, SCH], F32)
                    for h in range(2):
                        for k in range(KE):
                            nc.tensor.matmul(
                                pe[:, h * 512:(h + 1) * 512],
                                weT[:, k * D + m * P: k * D + (m + 1) * P],
                                enct_tiles[k][:, h * 512:(h + 1) * 512],
                                start=(k == 0),
                                stop=(k == KE - 1),
                            )
                    th = tanh_pool.tile([P, SCH], BF16)
                    nc.scalar.activation(
                        th[:], pe[:], AF.Tanh,
                        bias=biasT[:, b * KD + m: b * KD + m + 1],
                    )
                    tanh_tiles.append(th)

                # stage D: score chunk = v . tanh  (M=1 matmuls, accumulate over d)
                for h in range(2):
                    pv = v_psum.tile([1, 512], F32, tag="vrow")
                    for m in range(KD):
                        nc.tensor.matmul(
                            pv[:],
                            vT[:, m:m + 1],
                            tanh_tiles[m][:, h * 512:(h + 1) * 512],
                            start=(m == 0),
                            stop=(m == KD - 1),
                        )
                    nc.vector.tensor_copy(
                        score_row[0:1, c * SCH + h * 512: c * SCH + (h + 1) * 512],
                        pv[:],
                    )

            # ---- softmax over the full row ----
            mrow = mrow_pool.tile([1, S], F32)
            nc.sync.dma_start(mrow[:], mneg_in[b:b + 1, :])
            nc.vector.tensor_add(score_row[:], score_row[:], mrow[:])

            mx = tiny_pool.tile([1, 1], F32)
            nc.vector.reduce_max(mx[:], score_row[:], axis=AX.X)
            nmx = tiny_pool.tile([1, 1], F32)
            nc.vector.tensor_scalar_mul(nmx[:], mx[:], -1.0)
            nc.scalar.activation(score_row[:], score_row[:], AF.Exp, bias=nmx[:])

            ssum = tiny_pool.tile([1, 1], F32)
            nc.vector.reduce_sum(ssum[:], score_row[:], axis=AX.X)
            rec = tiny_pool.tile([1, 1], F32)
            nc.vector.reciprocal(rec[:], ssum[:])
            nc.vector.tensor_scalar_mul(score_row[:], score_row[:], rec[:])

            # attention weights out + bounce to get [128, 16] column layout
            nc.sync.dma_start(att_out[b:b + 1, :], score_row[:])
            dbounce = dram_pool.tile([1, S], F32)
            nc.sync.dma_start(dbounce[:], score_row[:])
            wcol = wcol_pool.tile([P, NT], F32R if USE_F32R else F32)
            nc.sync.dma_start(wcol[:], _r(dbounce[:].rearrange("a (j p) -> p (a j)", p=P)))

            # ---- context = sum_s w_s * enc[s, :] from resident natural tiles ----
            ctx_row = row_pool.tile([1, E], F32, tag="ctxrow")
            for h in range(2):
                pc = v_psum.tile([1, 512], F32, tag="vrow")
                for t in range(NT):
                    nc.tensor.matmul(
                        pc[:],
                        wcol[:, t:t + 1],
                        nat_tiles[t][:, h * 512:(h + 1) * 512],
                        start=(t == 0),
                        stop=(t == NT - 1),
                    )
                nc.vector.tensor_copy(ctx_row[0:1, h * 512:(h + 1) * 512], pc[:])
            nc.sync.dma_start(ctx_out[b:b + 1, :], ctx_row[:])

    nc.compile()
    return nc


_NC_CACHE = None


def _get_nc():
    global _NC_CACHE
    if _NC_CACHE is None:
        _NC_CACHE = build_nc()
    return _NC_CACHE


def _prep_inputs(dec_hidden, encoder_outputs, mask, W_w, W_b, v_w):
    dh = np.asarray(dec_hidden, dtype=np.float32)[0]            # [64, 1024]
    enc = np.asarray(encoder_outputs, dtype=np.float32)         # [64, 2048, 1024]
    mk = np.asarray(mask)                                       # [64, 1, 2048] i32
    Ww = np.asarray(W_w, dtype=np.float32)
    Wb = np.asarray(W_b, dtype=np.float32)
    vw = np.asarray(v_w, dtype=np.float32)

    Wd = Ww[:, :D]
    We = Ww[:, D:]
    bias_all = (dh @ Wd.T + Wb).astype(np.float32)              # [64, 1024]
    WeT = np.ascontiguousarray(We.T).astype(ml_dtypes.bfloat16)  # [1024, 1024]
    vb = vw.astype(ml_dtypes.bfloat16)                          # [1, 1024]
    mneg = np.where(mk[:, 0, :] == 0, np.float32(-1e10),
                    np.float32(0.0)).astype(np.float32)         # [64, 2048]

    in_maps = []
    for c in range(NCORES):
        sl = slice(c * BPC, (c + 1) * BPC)
        in_maps.append({
            "enc_in": np.ascontiguousarray(enc[sl]),
            "wet_in": WeT,
            "bias_in": np.ascontiguousarray(bias_all[sl]),
            "vb_in": vb,
            "mneg_in": np.ascontiguousarray(mneg[sl]),
            "ident_in": np.eye(P, dtype=np.float32),
        })
    return in_maps


def kernel(dec_hidden, encoder_outputs, mask, W_w, W_b, v_w):
    global LAST_RESULTS
    in_maps = _prep_inputs(dec_hidden, encoder_outputs, mask, W_w, W_b, v_w)
    nc = _get_nc()
    res = run_bass_kernel_spmd(nc, in_maps, core_ids=list(range(NCORES)),
                               trace=TRACE)
    LAST_RESULTS = res

    context = np.empty((B, 1, E), dtype=np.float32)
    atten = np.empty((B, S), dtype=np.float32)
    for c in range(NCORES):
        context[c * BPC:(c + 1) * BPC, 0, :] = res.results[c]["ctx_out"]
        atten[c * BPC:(c + 1) * BPC, :] = res.results[c]["att_out"]
    return context, atten


def bench(dec_hidden, encoder_outputs, mask, W_w, W_b, v_w, iters=20):
    """Steady-state device execution timing: device-resident inputs, repeated
    execution of the sharded NEFF body, wall time per iteration."""
    import time

    import jax
    import jax.numpy as jnp
    from jax.experimental.shard_map import shard_map
    from jax.sharding import Mesh, PartitionSpec

    from concourse import bass2jax, mybir as mb

    bass2jax.install_neuronx_cc_hook()
    nc = _get_nc()
    in_maps = _prep_inputs(dec_hidden, encoder_outputs, mask, W_w, W_b, v_w)

    partition_name = (nc.partition_id_tensor.name
                      if nc.partition_id_tensor else None)
    in_names, out_names, out_avals, zero_outs = [], [], [], []
    for alloc in nc.m.functions[0].allocations:
        if not isinstance(alloc, mb.MemoryLocationSet):
            continue
        name = alloc.memorylocations[0].name
        if alloc.kind == "ExternalInput":
            if name != partition_name:
                in_names.append(name)
        elif alloc.kind == "ExternalOutput":
            out_names.append(name)
            shape = tuple(alloc.tensor_shape)
            dtype = mb.dt.np(alloc.dtype)
            out_avals.append(jax.core.ShapedArray(shape, dtype))
            zero_outs.append(np.zeros(shape, dtype))
    n_params = len(in_names)
    all_names = in_names + out_names
    if partition_name is not None:
        all_names = all_names + [partition_name]

    def _body(*args):
        operands = list(args)
        if partition_name is not None:
            operands.append(bass2jax.partition_id_tensor())
        outs = bass2jax._bass_exec_p.bind(
            *operands,
            out_avals=tuple(out_avals),
            in_names=tuple(all_names),
            out_names=tuple(out_names),
            lowering_input_output_aliases=(),
            sim_require_finite=True,
            sim_require_nnan=True,
            nc=nc,
        )
        return tuple(outs)

    devices = jax.devices()[:NCORES]
    mesh = Mesh(np.asarray(devices), ("core",))
    spec = PartitionSpec("core")
    fn = jax.jit(shard_map(
        _body, mesh=mesh,
        in_specs=(spec,) * (n_params + len(out_names)),
        out_specs=(spec,) * len(out_names),
        check_rep=False,
    ))

    sharding = jax.sharding.NamedSharding(mesh, spec)
    dev_args = []
    for i, name in enumerate(in_names):
        arr = np.concatenate([np.asarray(m[name]) for m in in_maps], axis=0)
        dev_args.append(jax.device_put(arr, sharding))
    for z in zero_outs:
        arr = np.concatenate([z] * NCORES, axis=0)
        dev_args.append(jax.device_put(arr, sharding))

    out = fn(*dev_args)
    jax.block_until_ready(out)          # compile + warm
    t0 = time.time()
    for _ in range(iters):
        out = fn(*dev_args)
    jax.block_until_ready(out)
    dt = (time.time() - t0) / iters
    return dt
